# revision 12
# baseline (speedup 1.0000x reference)
"""ComputeAlignmentError kernel for 8 TRN2 NeuronCores.

Math: for each batch b, pairwise alignment error
    err[i,j] = || Ep_j (pc_i - bp_j) - Et_j (tc_i - bt_j) + eps ||_2
where Ep/Et are orthonormal frame bases built from pred/true frames and
bp/bt are the frame origins.  Because Ep/Et are rotations, this collapses
into a rank-18 bilinear form  err^2[i,j] = Y[i] . Z[j]  with
    Y[i] = [1, |pc|^2, |tc|^2, pc, tc, vec(pc tc^T)] * mask_i     (18)
    Z[j] = [z0 + delta, 1, 1, -2(bp - R bt), -2(bt - R^T bp),
            -2 vec(R)] * mask_j                                   (18)
    R_j = Ep_j^T Et_j,  z0 = |bp|^2 + |bt|^2 - 2 bp.(R bt)
The 1e-8 eps terms of the reference are dropped (contribute ~1e-7 to
err^2; tolerance is 2e-2).  `delta` (~1e-2) keeps err^2 positive under
float32r matmul rounding so the ACT sqrt never sees a negative input;
it biases err by delta/(2 err) ~ 2e-3 absolute, well inside tolerance.
Masked entries stay exactly 0: the mask multiplies every Z/Y slot
(including z0+delta), so err^2 = 0 -> sqrt = 0.

Each core handles one (batch, 512-row i-slice): builds Z for all 2048 j
and Y for its 512 i on-chip, transposes both feature-major via the PE
(strided chunk grouping so each [18,512] rhs block lands on one 32-row
partition band), then per (i-tile, half): 2 f32r matmuls [18,128]x[18,512]
-> PSUM, one ACT sqrt pass PSUM->SBUF, one 512 KB DMA.  Dummy warm-up
transposes keep the PE clock-gate (HAM) warm through the feature phase.
"""

import sys

import numpy as np

sys.path.insert(0, "/opt/trn_rl_repo")

from contextlib import ExitStack

import concourse.bacc as bacc
import concourse.bass as bass
import concourse.tile as tile
from concourse import mybir
from concourse.bass_utils import run_bass_kernel_spmd
from concourse.masks import make_identity

F32 = mybir.dt.float32
F32R = mybir.dt.float32r
ALU = mybir.AluOpType

B, N = 2, 2048
NCORES = 8
ISLICE = N * B // NCORES  # 512 rows of i per core
NITILE = ISLICE // 128  # 4 i-tiles per core
NJCH = N // 128  # 16 j-chunks
NF = 18  # feature count K
FPAD = 32  # feature slot padding (partition-band alignment)
G = 2 * NJCH  # 32 (set, chunk) groups

DELTA = 1e-2  # err^2 positivity margin under f32r rounding
NWARM = 20  # PE warm-up transposes gated on the feature phase
NFILL = 5  # PE filler transposes between output half-tiles
HALF = 1024  # output processed in [128, HALF] half-tiles

# Zb slot q holds j-chunk CHUNK_PERM[q] = (q // 4) + 4 * (q % 4) (self-inverse)
CHUNK_PERM = [(q // 4) + 4 * (q % 4) for q in range(NJCH)]


def _build(nc_holder=[]):
    if nc_holder:
        return nc_holder[0]
    nc = bacc.Bacc(
        "TRN2",
        target_bir_lowering=False,
        debug=False,
        enable_asserts=True,
        num_devices=NCORES,
    )
    NIN = 2 * NJCH * 9 + NITILE * 6 + NJCH + NITILE  # 332
    all_in = nc.dram_tensor("allin", [128, NIN], F32, kind="ExternalInput").ap()
    out_dram = nc.dram_tensor("out", [ISLICE, N], F32, kind="ExternalOutput").ap()

    with tile.TileContext(nc) as tc, ExitStack() as ctx:
        _kernel_body(ctx, tc, out_dram, all_in)

    nc.compile()
    nc_holder.append(nc)
    return nc


def _kernel_body(ctx, tc, out_dram, all_in):
    nc = tc.nc
    P = 128
    sb = ctx.enter_context(tc.tile_pool(name="sb", bufs=1))
    outp = ctx.enter_context(tc.tile_pool(name="outp", bufs=3))
    tp = ctx.enter_context(tc.tile_pool(name="tp", bufs=2, space="PSUM"))
    pmp = ctx.enter_context(tc.tile_pool(name="pmp", bufs=3, space="PSUM"))

    # ---- one merged input DMA (single completion semaphore) ---------------
    NIN = 2 * NJCH * 9 + NITILE * 6 + NJCH + NITILE
    Ain = sb.tile([P, NIN], F32, tag="Ain")
    nc.sync.dma_start(out=Ain[:], in_=all_in[:])
    o0 = 0
    Ft = Ain[:, o0 : o0 + 2 * NJCH * 9].rearrange(
        "p (s c t x) -> p s c t x", s=2, c=NJCH, t=3
    )
    o0 += 2 * NJCH * 9
    Ct = Ain[:, o0 : o0 + NITILE * 6].rearrange("p (c s x) -> p c s x", c=NITILE, s=2)
    o0 += NITILE * 6
    Mj = Ain[:, o0 : o0 + NJCH]
    o0 += NJCH
    Mi = Ain[:, o0 : o0 + NITILE]

    # ---- t=0 setup: ACT table trigger, identity, PE warm-up ---------------
    scr = sb.tile([P, 2], F32, tag="scr")
    nc.gpsimd.memset(scr[:, 0:1], 1.0)
    # first ACT op is a sqrt -> loads sqrt_and_others once (covers square/
    # copy/identity too); runs under the input-DMA wait
    nc.scalar.sqrt(scr[:, 1:2], scr[:, 0:1])

    ident = sb.tile([P, P], F32, tag="ident")
    make_identity(nc, ident[:])

    # ---- early independent ops --------------------------------------------
    Zb = sb.tile([P, NJCH, FPAD], F32, tag="Zb")
    nc.gpsimd.tensor_copy(
        Zb[:, :, 1:3], Mj.unsqueeze(2).broadcast_to((P, NJCH, 2))
    )
    # z0 partials: [bp, bt]^2 -> H9[0:6] (needs only frames)
    bb = Ft[:, :, :, 1, :].transpose([0, 2, 1, 3])  # [p, c, set, xyz]
    H9 = sb.tile([P, NJCH, 9], F32, tag="H9")
    nc.scalar.square(H9[:, :, 0:6].rearrange("p c (s x) -> p c s x", s=2), bb)

    # ---- Y features (band-replicated for the 4 PE row groups) -------------
    YbR = sb.tile([P, NITILE, 4, FPAD], F32, tag="YbR")  # [p, chunk, band, f]
    Cm = sb.tile([P, NITILE, 2, 3], F32, tag="Cm")
    nc.gpsimd.tensor_mul(
        Cm[:], Ct, Mi.unsqueeze(2).unsqueeze(3).broadcast_to((P, NITILE, 2, 3))
    )
    sqc = sb.tile([P, NITILE, 2, 3], F32, tag="sqc")
    nc.gpsimd.tensor_mul(sqc[:], Cm[:], Ct)
    nc.gpsimd.tensor_copy(YbR[:, :, 0, 0:1], Mi.unsqueeze(2))
    nc.gpsimd.tensor_copy(
        YbR[:, :, 0, 3:9], Cm[:].rearrange("p c s x -> p c (s x)")
    )
    nc.vector.reduce_sum(
        YbR[:, :, 0, 1:3].unsqueeze(3), sqc[:], axis=mybir.AxisListType.X
    )
    nc.vector.tensor_mul(
        YbR[:, :, 0, 9:18].rearrange("p c (a b) -> p c a b", a=3),
        Cm[:, :, 0, :].unsqueeze(3).broadcast_to((P, NITILE, 3, 3)),
        Ct[:, :, 1, :].unsqueeze(2).broadcast_to((P, NITILE, 3, 3)),
    )
    nc.vector.tensor_copy(
        YbR[:, :, 1:4, 0:NF],
        YbR[:, :, 0, 0:NF].unsqueeze(2).broadcast_to((P, NITILE, 3, NF)),
    )

    # Y transposes: in_ = [p, (band, f)] per chunk -> PSUM bands all hold
    # that chunk's features; one [128,128] copy per chunk into YTrep
    YTrep = sb.tile([P, NITILE * P], F32R, tag="YTrep")
    for t in range(NITILE):
        pt = tp.tile([P, P], F32, tag="tp")
        nc.tensor.transpose(
            pt[:], YbR[:, t, :, :].rearrange("p b f -> p (b f)"), ident[:]
        )
        eng = nc.scalar if t % 2 == 0 else nc.vector
        if t % 2 == 0:
            nc.scalar.copy(YTrep[:, t * P : (t + 1) * P], pt[:])
        else:
            nc.vector.tensor_copy(YTrep[:, t * P : (t + 1) * P], pt[:])

    # ---- frame bases (both sets, all j-chunks at once) --------------------
    Fg = Ft.rearrange("p s c t x -> p (s c) t x")  # [p, g, pt, xyz]
    w12 = sb.tile([P, G, 2, 3], F32, tag="w12")  # w1 = a - b, w2 = c - b
    nc.vector.tensor_sub(
        w12[:],
        Fg[:, :, 0::2, :],
        Fg[:, :, 1, :].unsqueeze(2).broadcast_to((P, G, 2, 3)),
    )
    sq1 = sb.tile([P, G, 2, 3], F32, tag="sq1")
    nc.vector.tensor_mul(sq1[:], w12[:], w12[:])
    n2 = sb.tile([P, G, 2], F32, tag="n2")
    nc.vector.reduce_sum(n2[:].unsqueeze(3), sq1[:], axis=mybir.AxisListType.X)
    nrm = sb.tile([P, G, 2], F32, tag="nrm")
    nc.scalar.sqrt(nrm[:], n2[:])
    rinv = sb.tile([P, G, 2], F32, tag="rinv")
    nc.vector.reciprocal_approx_fast(rinv[:], nrm[:])
    w12n = sb.tile([P, G, 2, 3], F32, tag="w12n")
    nc.vector.tensor_mul(
        w12n[:], w12[:], rinv[:].unsqueeze(3).broadcast_to((P, G, 2, 3))
    )

    e12p = sb.tile([P, G, 2, 3], F32, tag="e12p")
    nc.vector.tensor_add(e12p[:, :, 0, :], w12n[:, :, 0, :], w12n[:, :, 1, :])
    nc.vector.tensor_sub(e12p[:, :, 1, :], w12n[:, :, 1, :], w12n[:, :, 0, :])
    sq2 = sb.tile([P, G, 2, 3], F32, tag="sq2")
    nc.vector.tensor_mul(sq2[:], e12p[:], e12p[:])
    n2b = sb.tile([P, G, 2], F32, tag="n2b")
    nc.vector.reduce_sum(n2b[:].unsqueeze(3), sq2[:], axis=mybir.AxisListType.X)
    nrmb = sb.tile([P, G, 2], F32, tag="nrmb")
    nc.scalar.sqrt(nrmb[:], n2b[:])
    rinvb = sb.tile([P, G, 2], F32, tag="rinvb")
    nc.vector.reciprocal_approx_fast(rinvb[:], nrmb[:])

    # Est[p, g, k, xyz]: rows e1, e2 normalized; e3 = e1 x e2
    Est = sb.tile([P, G, 3, 3], F32, tag="Est")
    nc.vector.tensor_mul(
        Est[:, :, 0:2, :],
        e12p[:],
        rinvb[:].unsqueeze(3).broadcast_to((P, G, 2, 3)),
    )
    cbuf = sb.tile([P, G, 2, 6], F32, tag="cbuf")  # duplicated for rotations
    nc.vector.tensor_copy(cbuf[:, :, :, 0:3], Est[:, :, 0:2, :])
    nc.scalar.copy(cbuf[:, :, :, 3:6], Est[:, :, 0:2, :])
    mtmp = sb.tile([P, G, 2, 3], F32, tag="mtmp")
    nc.vector.tensor_mul(mtmp[:, :, 0, :], cbuf[:, :, 0, 1:4], cbuf[:, :, 1, 2:5])
    nc.vector.tensor_mul(mtmp[:, :, 1, :], cbuf[:, :, 0, 2:5], cbuf[:, :, 1, 1:4])
    nc.vector.tensor_sub(Est[:, :, 2, :], mtmp[:, :, 0, :], mtmp[:, :, 1, :])

    # ---- Z features -------------------------------------------------------
    Estv = Est[:].rearrange("p (s c) k x -> p s c k x", s=2)
    Ep = Estv[:, 0]  # [p, c, k, xyz]
    Et_ = Estv[:, 1]
    bp = Ft[:, 0, :, 1, :]  # [p, c, xyz]
    bt = Ft[:, 1, :, 1, :]

    # R[c, a, b] = sum_k Ep[c,k,a] * Et[c,k,b]; EtT materialized once so the
    # three muls read a contiguous in1 (transposed APs run ~2x slower on DVE)
    EtT = sb.tile([P, NJCH, 3, 3], F32, tag="EtT")  # [c, xyz, k]
    nc.gpsimd.tensor_copy(EtT[:], Et_.transpose([0, 1, 3, 2]))
    prodR = sb.tile([P, NJCH, 9, 3], F32, tag="prodR")  # [c, (a b), k]
    for a in range(3):
        eng = nc.gpsimd if a == 2 else nc.vector
        eng.tensor_mul(
            prodR[:, :, 3 * a : 3 * a + 3, :],
            Ep[:, :, :, a].unsqueeze(2).broadcast_to((P, NJCH, 3, 3)),
            EtT[:],
        )
    Rb = sb.tile([P, NJCH, 9], F32, tag="Rb")
    nc.vector.reduce_sum(Rb[:].unsqueeze(3), prodR[:], axis=mybir.AxisListType.X)
    Rbv = Rb[:].rearrange("p c (a b) -> p c a b", a=3)

    # Rbt[c,a] = sum_b R[c,a,b] bt[c,b] ; Rtbp[c,b] = sum_a R[c,a,b] bp[c,a]
    prodv = sb.tile([P, NJCH, 6, 3], F32, tag="prodv")
    nc.vector.tensor_mul(
        prodv[:, :, 0:3, :], Rbv, bt.unsqueeze(2).broadcast_to((P, NJCH, 3, 3))
    )
    nc.vector.tensor_mul(
        prodv[:, :, 3:6, :],
        Rbv.transpose([0, 1, 3, 2]),
        bp.unsqueeze(2).broadcast_to((P, NJCH, 3, 3)),
    )
    Rv = sb.tile([P, NJCH, 6], F32, tag="Rv")  # [Rbt | Rtbp]
    nc.vector.reduce_sum(Rv[:].unsqueeze(3), prodv[:], axis=mybir.AxisListType.X)

    Zb = Zb  # [P, NJCH, FPAD]; slots 1:3 already hold Mj
    t2 = sb.tile([P, NJCH, 2, 3], F32, tag="t2")
    nc.vector.tensor_sub(t2[:], bb, Rv[:].rearrange("p c (v x) -> p c v x", v=2))
    # zp|zt = -2 (b - Rv) * mask
    nc.vector.scalar_tensor_tensor(
        Zb[:, :, 3:9],
        t2[:].rearrange("p c s x -> p c (s x)"),
        -2.0,
        Mj.unsqueeze(2).broadcast_to((P, NJCH, 6)),
        op0=ALU.mult,
        op1=ALU.mult,
    )
    # -2 R * mask
    nc.vector.scalar_tensor_tensor(
        Zb[:, :, 9:18],
        Rb[:],
        -2.0,
        Mj.unsqueeze(2).broadcast_to((P, NJCH, 9)),
        op0=ALU.mult,
        op1=ALU.mult,
    )
    # z0 = |bp|^2 + |bt|^2 - 2 bp.Rbt ; slot0 = (z0 + delta) * mask
    nc.vector.scalar_tensor_tensor(
        H9[:, :, 6:9], bp, -2.0, Rv[:, :, 0:3], op0=ALU.mult, op1=ALU.mult
    )
    z0 = sb.tile([P, NJCH], F32, tag="z0")
    nc.vector.reduce_sum(z0[:].unsqueeze(2), H9[:], axis=mybir.AxisListType.X)
    nc.vector.scalar_tensor_tensor(
        Zb[:, :, 0:1],
        z0[:].unsqueeze(2),
        DELTA,
        Mj.unsqueeze(2),
        op0=ALU.add,
        op1=ALU.mult,
    )

    # ---- PE warm-up: dense transpose block gated on sq2 so the HAM
    # clock-gate sees sustained activity right before the matmul burst
    for w in range(NWARM):
        wt = tp.tile([P, P], F32, tag="tp")
        nc.tensor.transpose(
            wt[:], sq2[:].rearrange("p g e x -> p (g e x)")[:, 0:P], ident[:]
        )

    # ---- Z transposes -> band-coherent ZT ---------------------------------
    # The host stores j-chunk (g + 4k) in Zb slot (4g + k), so transpose g
    # reads a contiguous slot group [4g, 4g+4) and PSUM band k ends up with
    # chunk g+4k: ZT band jb spans j [512 jb, 512 jb + 512) contiguously.
    ZT = sb.tile([P, NITILE * P], F32R, tag="ZT")
    for g in range(4):
        pz = tp.tile([P, P], F32, tag="tp")
        nc.tensor.transpose(
            pz[:], Zb[:, 4 * g : 4 * g + 4, :].rearrange("p c f -> p (c f)"), ident[:]
        )
        if g % 2 == 0:
            nc.scalar.copy(ZT[:, g * P : (g + 1) * P], pz[:])
        else:
            nc.vector.tensor_copy(ZT[:, g * P : (g + 1) * P], pz[:])

    # ---- main: matmul + sqrt + DMA out ------------------------------------
    for it in range(NITILE):
        for h in range(N // HALF):
            pm = pmp.tile([P, HALF], F32, tag="pm")
            for jj in range(HALF // 512):
                jb = (HALF // 512) * h + jj
                nc.tensor.matmul(
                    pm[:, jj * 512 : (jj + 1) * 512],
                    YTrep[32 * jb : 32 * jb + NF, it * P : (it + 1) * P],
                    ZT[32 * jb : 32 * jb + NF, :],
                    start=True,
                    stop=True,
                    tile_position=(32 * jb, 0),
                )
            ot = outp.tile([P, HALF], F32, tag="ot")
            nc.scalar.sqrt(ot[:], pm[:])
            nc.sync.dma_start(
                out=out_dram[it * P : (it + 1) * P, h * HALF : (h + 1) * HALF],
                in_=ot[:],
            )
            if it * 2 + h < 7:
                # keep the PE continuously busy between half-tiles so its
                # clock stays ramped (idle gaps re-throttle it)
                for w in range(NFILL):
                    wt = tp.tile([P, P], F32, tag="tp")
                    nc.tensor.transpose(wt[:], ident[:], ident[:])


def _shard_inputs(pred_coords, true_coords, pred_frames, true_frames, mask):
    """Host-side reformat into per-core DMA-friendly layouts."""
    pc = np.asarray(pred_coords, np.float32)
    tc = np.asarray(true_coords, np.float32)
    pf = np.asarray(pred_frames, np.float32)
    tf = np.asarray(true_frames, np.float32)
    mk = np.asarray(mask).astype(np.float32)

    in_maps = []
    for core in range(NCORES):
        b = core // (NCORES // B)
        i0 = (core % (NCORES // B)) * ISLICE
        # frames [128, set, c, pt, xyz] ; input frames are [n, xyz, pt].
        # j-chunk columns are permuted so Zb slot 4g+k holds chunk g+4k:
        # the device's contiguous transpose groups then produce partition
        # bands covering contiguous 512-j blocks.
        fr = np.stack([pf[b], tf[b]], axis=0)  # [2, n, 3xyz, 3pt]
        fr = fr.transpose(0, 1, 3, 2)  # [2, n, pt, xyz]
        fr = fr.reshape(2, NJCH, 128, 3, 3)[:, CHUNK_PERM]
        fr = fr.transpose(2, 0, 1, 3, 4)
        frames = np.ascontiguousarray(fr.reshape(128, -1))
        # coords [128, chunk, set, xyz]
        co = np.stack([pc[b, i0 : i0 + ISLICE], tc[b, i0 : i0 + ISLICE]], axis=1)
        co = co.reshape(NITILE, 128, 2, 3).transpose(1, 0, 2, 3)
        coords = np.ascontiguousarray(co.reshape(128, -1))
        maskj = np.ascontiguousarray(mk[b].reshape(NJCH, 128)[CHUNK_PERM].T)
        maski = np.ascontiguousarray(
            mk[b, i0 : i0 + ISLICE].reshape(NITILE, 128).T
        )
        allin = np.ascontiguousarray(
            np.concatenate([frames, coords, maskj, maski], axis=1)
        )
        in_maps.append({"allin": allin})
    return in_maps


def kernel(pred_coords, true_coords, pred_frames, true_frames, mask, _res=[]):
    nc = _build()
    in_maps = _shard_inputs(pred_coords, true_coords, pred_frames, true_frames, mask)
    res = run_bass_kernel_spmd(nc, in_maps, list(range(NCORES)))
    _res.clear()
    _res.append(res)
    out = np.empty((B, N, N), np.float32)
    for core in range(NCORES):
        b = core // (NCORES // B)
        i0 = (core % (NCORES // B)) * ISLICE
        out[b, i0 : i0 + ISLICE, :] = res.results[core]["out"]
    return out


if __name__ == "__main__":
    rng = np.random.default_rng(0)
    ins = {
        "pred_coords": rng.standard_normal((B, N, 3)).astype(np.float32),
        "true_coords": rng.standard_normal((B, N, 3)).astype(np.float32),
        "pred_frames": rng.standard_normal((B, N, 3, 3)).astype(np.float32),
        "true_frames": rng.standard_normal((B, N, 3, 3)).astype(np.float32),
        "mask": np.ones((B, N), bool),
    }
    out = kernel(**ins)
    print("out", out.shape, out.dtype, float(np.abs(out).max()))


# revision 13
# speedup vs baseline: 1.0545x; 1.0545x over previous
"""ComputeAlignmentError kernel for 8 TRN2 NeuronCores.

Math: for each batch b, pairwise alignment error
    err[i,j] = || Ep_j (pc_i - bp_j) - Et_j (tc_i - bt_j) + eps ||_2
where Ep/Et are orthonormal frame bases built from pred/true frames and
bp/bt are the frame origins.  Because Ep/Et are rotations, this collapses
into a rank-18 bilinear form  err^2[i,j] = Y[i] . Z[j]  with
    Y[i] = [1, |pc|^2, |tc|^2, pc, tc, vec(pc tc^T)] * mask_i     (18)
    Z[j] = [z0 + delta, 1, 1, -2(bp - R bt), -2(bt - R^T bp),
            -2 vec(R)] * mask_j                                   (18)
    R_j = Ep_j^T Et_j,  z0 = |bp|^2 + |bt|^2 - 2 bp.(R bt)
The 1e-8 eps terms of the reference are dropped (contribute ~1e-7 to
err^2; tolerance is 2e-2).  `delta` (~1e-2) keeps err^2 positive under
float32r matmul rounding so the ACT sqrt never sees a negative input;
it biases err by delta/(2 err) ~ 2e-3 absolute, well inside tolerance.
Masked entries stay exactly 0: the mask multiplies every Z/Y slot
(including z0+delta), so err^2 = 0 -> sqrt = 0.

Each core handles one (batch, 512-row i-slice): builds Z for all 2048 j
and Y for its 512 i on-chip, transposes both feature-major via the PE
(strided chunk grouping so each [18,512] rhs block lands on one 32-row
partition band), then per (i-tile, half): 2 f32r matmuls [18,128]x[18,512]
-> PSUM, one ACT sqrt pass PSUM->SBUF, one 512 KB DMA.  Dummy warm-up
transposes keep the PE clock-gate (HAM) warm through the feature phase.
"""

import sys

import numpy as np

sys.path.insert(0, "/opt/trn_rl_repo")

from contextlib import ExitStack

import concourse.bacc as bacc
import concourse.bass as bass
import concourse.tile as tile
from concourse import mybir
from concourse.bass_utils import run_bass_kernel_spmd
from concourse.masks import make_identity

F32 = mybir.dt.float32
F32R = mybir.dt.float32r
ALU = mybir.AluOpType

B, N = 2, 2048
NCORES = 8
ISLICE = N * B // NCORES  # 512 rows of i per core
NITILE = ISLICE // 128  # 4 i-tiles per core
NJCH = N // 128  # 16 j-chunks
NF = 18  # feature count K
FPAD = 32  # feature slot padding (partition-band alignment)
G = 2 * NJCH  # 32 (set, chunk) groups

DELTA = 1e-2  # err^2 positivity margin under f32r rounding
NWARM = 20  # PE warm-up transposes gated on the feature phase
NFILL = 1  # PE filler transposes between output half-tiles
HALF = 1024  # output processed in [128, HALF] half-tiles

# Zb slot q holds j-chunk CHUNK_PERM[q] = (q // 4) + 4 * (q % 4) (self-inverse)
CHUNK_PERM = [(q // 4) + 4 * (q % 4) for q in range(NJCH)]


def _build(nc_holder=[]):
    if nc_holder:
        return nc_holder[0]
    nc = bacc.Bacc(
        "TRN2",
        target_bir_lowering=False,
        debug=False,
        enable_asserts=True,
        num_devices=NCORES,
    )
    NIN = 2 * NJCH * 9 + NITILE * 6 + NJCH + NITILE  # 332
    all_in = nc.dram_tensor("allin", [128, NIN], F32, kind="ExternalInput").ap()
    out_dram = nc.dram_tensor("out", [ISLICE, N], F32, kind="ExternalOutput").ap()

    with tile.TileContext(nc) as tc, ExitStack() as ctx:
        _kernel_body(ctx, tc, out_dram, all_in)

    nc.compile()
    nc_holder.append(nc)
    return nc


def _kernel_body(ctx, tc, out_dram, all_in):
    nc = tc.nc
    P = 128
    sb = ctx.enter_context(tc.tile_pool(name="sb", bufs=1))
    outp = ctx.enter_context(tc.tile_pool(name="outp", bufs=3))
    tp = ctx.enter_context(tc.tile_pool(name="tp", bufs=2, space="PSUM"))
    pmp = ctx.enter_context(tc.tile_pool(name="pmp", bufs=3, space="PSUM"))

    # ---- one merged input DMA (single completion semaphore) ---------------
    NIN = 2 * NJCH * 9 + NITILE * 6 + NJCH + NITILE
    Ain = sb.tile([P, NIN], F32, tag="Ain")
    nc.sync.dma_start(out=Ain[:], in_=all_in[:])
    o0 = 0
    Ft = Ain[:, o0 : o0 + 2 * NJCH * 9].rearrange(
        "p (s c t x) -> p s c t x", s=2, c=NJCH, t=3
    )
    o0 += 2 * NJCH * 9
    Ct = Ain[:, o0 : o0 + NITILE * 6].rearrange("p (c s x) -> p c s x", c=NITILE, s=2)
    o0 += NITILE * 6
    Mj = Ain[:, o0 : o0 + NJCH]
    o0 += NJCH
    Mi = Ain[:, o0 : o0 + NITILE]

    # ---- t=0 setup: ACT table trigger, identity, PE warm-up ---------------
    scr = sb.tile([P, 2], F32, tag="scr")
    nc.gpsimd.memset(scr[:, 0:1], 1.0)
    # first ACT op is a sqrt -> loads sqrt_and_others once (covers square/
    # copy/identity too); runs under the input-DMA wait
    nc.scalar.sqrt(scr[:, 1:2], scr[:, 0:1])

    ident = sb.tile([P, P], F32, tag="ident")
    make_identity(nc, ident[:])

    # ---- early independent ops --------------------------------------------
    Zb = sb.tile([P, NJCH, FPAD], F32, tag="Zb")
    nc.gpsimd.tensor_copy(
        Zb[:, :, 1:3], Mj.unsqueeze(2).broadcast_to((P, NJCH, 2))
    )
    # z0 partials: [bp, bt]^2 -> H9[0:6] (needs only frames)
    bb = Ft[:, :, :, 1, :].transpose([0, 2, 1, 3])  # [p, c, set, xyz]
    H9 = sb.tile([P, NJCH, 9], F32, tag="H9")
    nc.scalar.square(H9[:, :, 0:6].rearrange("p c (s x) -> p c s x", s=2), bb)

    # ---- Y features (band-replicated for the 4 PE row groups) -------------
    YbR = sb.tile([P, NITILE, 4, FPAD], F32, tag="YbR")  # [p, chunk, band, f]
    Cm = sb.tile([P, NITILE, 2, 3], F32, tag="Cm")
    nc.gpsimd.tensor_mul(
        Cm[:], Ct, Mi.unsqueeze(2).unsqueeze(3).broadcast_to((P, NITILE, 2, 3))
    )
    sqc = sb.tile([P, NITILE, 2, 3], F32, tag="sqc")
    nc.gpsimd.tensor_mul(sqc[:], Cm[:], Ct)
    nc.gpsimd.tensor_copy(YbR[:, :, 0, 0:1], Mi.unsqueeze(2))
    nc.gpsimd.tensor_copy(
        YbR[:, :, 0, 3:9], Cm[:].rearrange("p c s x -> p c (s x)")
    )
    nc.vector.reduce_sum(
        YbR[:, :, 0, 1:3].unsqueeze(3), sqc[:], axis=mybir.AxisListType.X
    )
    nc.vector.tensor_mul(
        YbR[:, :, 0, 9:18].rearrange("p c (a b) -> p c a b", a=3),
        Cm[:, :, 0, :].unsqueeze(3).broadcast_to((P, NITILE, 3, 3)),
        Ct[:, :, 1, :].unsqueeze(2).broadcast_to((P, NITILE, 3, 3)),
    )
    nc.vector.tensor_copy(
        YbR[:, :, 1:4, 0:NF],
        YbR[:, :, 0, 0:NF].unsqueeze(2).broadcast_to((P, NITILE, 3, NF)),
    )

    # Y transposes: in_ = [p, (band, f)] per chunk -> PSUM bands all hold
    # that chunk's features; one [128,128] copy per chunk into YTrep
    YTrep = sb.tile([P, NITILE * P], F32R, tag="YTrep")
    for t in range(NITILE):
        pt = tp.tile([P, P], F32, tag="tp")
        nc.tensor.transpose(
            pt[:], YbR[:, t, :, :].rearrange("p b f -> p (b f)"), ident[:]
        )
        eng = nc.scalar if t % 2 == 0 else nc.vector
        if t % 2 == 0:
            nc.scalar.copy(YTrep[:, t * P : (t + 1) * P], pt[:])
        else:
            nc.vector.tensor_copy(YTrep[:, t * P : (t + 1) * P], pt[:])

    # ---- frame bases (both sets, all j-chunks at once) --------------------
    Fg = Ft.rearrange("p s c t x -> p (s c) t x")  # [p, g, pt, xyz]
    w12 = sb.tile([P, G, 2, 3], F32, tag="w12")  # w1 = a - b, w2 = c - b
    nc.vector.tensor_sub(
        w12[:],
        Fg[:, :, 0::2, :],
        Fg[:, :, 1, :].unsqueeze(2).broadcast_to((P, G, 2, 3)),
    )
    sq1 = sb.tile([P, G, 2, 3], F32, tag="sq1")
    nc.vector.tensor_mul(sq1[:], w12[:], w12[:])
    n2 = sb.tile([P, G, 2], F32, tag="n2")
    nc.vector.reduce_sum(n2[:].unsqueeze(3), sq1[:], axis=mybir.AxisListType.X)
    nrm = sb.tile([P, G, 2], F32, tag="nrm")
    nc.scalar.sqrt(nrm[:], n2[:])
    rinv = sb.tile([P, G, 2], F32, tag="rinv")
    nc.vector.reciprocal_approx_fast(rinv[:], nrm[:])
    w12n = sb.tile([P, G, 2, 3], F32, tag="w12n")
    nc.vector.tensor_mul(
        w12n[:], w12[:], rinv[:].unsqueeze(3).broadcast_to((P, G, 2, 3))
    )

    e12p = sb.tile([P, G, 2, 3], F32, tag="e12p")
    nc.vector.tensor_add(e12p[:, :, 0, :], w12n[:, :, 0, :], w12n[:, :, 1, :])
    nc.vector.tensor_sub(e12p[:, :, 1, :], w12n[:, :, 1, :], w12n[:, :, 0, :])
    sq2 = sb.tile([P, G, 2, 3], F32, tag="sq2")
    nc.vector.tensor_mul(sq2[:], e12p[:], e12p[:])
    n2b = sb.tile([P, G, 2], F32, tag="n2b")
    nc.vector.reduce_sum(n2b[:].unsqueeze(3), sq2[:], axis=mybir.AxisListType.X)
    nrmb = sb.tile([P, G, 2], F32, tag="nrmb")
    nc.scalar.sqrt(nrmb[:], n2b[:])
    rinvb = sb.tile([P, G, 2], F32, tag="rinvb")
    nc.vector.reciprocal_approx_fast(rinvb[:], nrmb[:])

    # Est[p, g, k, xyz]: rows e1, e2 normalized; e3 = e1 x e2
    Est = sb.tile([P, G, 3, 3], F32, tag="Est")
    nc.vector.tensor_mul(
        Est[:, :, 0:2, :],
        e12p[:],
        rinvb[:].unsqueeze(3).broadcast_to((P, G, 2, 3)),
    )
    cbuf = sb.tile([P, G, 2, 6], F32, tag="cbuf")  # duplicated for rotations
    nc.vector.tensor_copy(cbuf[:, :, :, 0:3], Est[:, :, 0:2, :])
    nc.scalar.copy(cbuf[:, :, :, 3:6], Est[:, :, 0:2, :])
    mtmp = sb.tile([P, G, 2, 3], F32, tag="mtmp")
    nc.vector.tensor_mul(mtmp[:, :, 0, :], cbuf[:, :, 0, 1:4], cbuf[:, :, 1, 2:5])
    nc.vector.tensor_mul(mtmp[:, :, 1, :], cbuf[:, :, 0, 2:5], cbuf[:, :, 1, 1:4])
    nc.vector.tensor_sub(Est[:, :, 2, :], mtmp[:, :, 0, :], mtmp[:, :, 1, :])

    # ---- Z features -------------------------------------------------------
    Estv = Est[:].rearrange("p (s c) k x -> p s c k x", s=2)
    Ep = Estv[:, 0]  # [p, c, k, xyz]
    Et_ = Estv[:, 1]
    bp = Ft[:, 0, :, 1, :]  # [p, c, xyz]
    bt = Ft[:, 1, :, 1, :]

    # R[c, a, b] = sum_k Ep[c,k,a] * Et[c,k,b]; EtT materialized once so the
    # three muls read a contiguous in1 (transposed APs run ~2x slower on DVE)
    EtT = sb.tile([P, NJCH, 3, 3], F32, tag="EtT")  # [c, xyz, k]
    nc.gpsimd.tensor_copy(EtT[:], Et_.transpose([0, 1, 3, 2]))
    prodR = sb.tile([P, NJCH, 9, 3], F32, tag="prodR")  # [c, (a b), k]
    for a in range(3):
        eng = nc.gpsimd if a == 2 else nc.vector
        eng.tensor_mul(
            prodR[:, :, 3 * a : 3 * a + 3, :],
            Ep[:, :, :, a].unsqueeze(2).broadcast_to((P, NJCH, 3, 3)),
            EtT[:],
        )
    Rb = sb.tile([P, NJCH, 9], F32, tag="Rb")
    nc.vector.reduce_sum(Rb[:].unsqueeze(3), prodR[:], axis=mybir.AxisListType.X)
    Rbv = Rb[:].rearrange("p c (a b) -> p c a b", a=3)

    # Rbt[c,a] = sum_b R[c,a,b] bt[c,b] ; Rtbp[c,b] = sum_a R[c,a,b] bp[c,a]
    prodv = sb.tile([P, NJCH, 6, 3], F32, tag="prodv")
    nc.vector.tensor_mul(
        prodv[:, :, 0:3, :], Rbv, bt.unsqueeze(2).broadcast_to((P, NJCH, 3, 3))
    )
    nc.vector.tensor_mul(
        prodv[:, :, 3:6, :],
        Rbv.transpose([0, 1, 3, 2]),
        bp.unsqueeze(2).broadcast_to((P, NJCH, 3, 3)),
    )
    Rv = sb.tile([P, NJCH, 6], F32, tag="Rv")  # [Rbt | Rtbp]
    nc.vector.reduce_sum(Rv[:].unsqueeze(3), prodv[:], axis=mybir.AxisListType.X)

    Zb = Zb  # [P, NJCH, FPAD]; slots 1:3 already hold Mj
    t2 = sb.tile([P, NJCH, 2, 3], F32, tag="t2")
    nc.vector.tensor_sub(t2[:], bb, Rv[:].rearrange("p c (v x) -> p c v x", v=2))
    # zp|zt = -2 (b - Rv) * mask
    nc.vector.scalar_tensor_tensor(
        Zb[:, :, 3:9],
        t2[:].rearrange("p c s x -> p c (s x)"),
        -2.0,
        Mj.unsqueeze(2).broadcast_to((P, NJCH, 6)),
        op0=ALU.mult,
        op1=ALU.mult,
    )
    # -2 R * mask
    nc.vector.scalar_tensor_tensor(
        Zb[:, :, 9:18],
        Rb[:],
        -2.0,
        Mj.unsqueeze(2).broadcast_to((P, NJCH, 9)),
        op0=ALU.mult,
        op1=ALU.mult,
    )
    # z0 = |bp|^2 + |bt|^2 - 2 bp.Rbt ; slot0 = (z0 + delta) * mask
    nc.vector.scalar_tensor_tensor(
        H9[:, :, 6:9], bp, -2.0, Rv[:, :, 0:3], op0=ALU.mult, op1=ALU.mult
    )
    z0 = sb.tile([P, NJCH], F32, tag="z0")
    nc.vector.reduce_sum(z0[:].unsqueeze(2), H9[:], axis=mybir.AxisListType.X)
    nc.vector.scalar_tensor_tensor(
        Zb[:, :, 0:1],
        z0[:].unsqueeze(2),
        DELTA,
        Mj.unsqueeze(2),
        op0=ALU.add,
        op1=ALU.mult,
    )

    # ---- PE warm-up: dense transpose block gated on sq2 so the HAM
    # clock-gate sees sustained activity right before the matmul burst
    for w in range(NWARM):
        wt = tp.tile([P, P], F32, tag="tp")
        nc.tensor.transpose(
            wt[:], sq2[:].rearrange("p g e x -> p (g e x)")[:, 0:P], ident[:]
        )

    # ---- Z transposes -> band-coherent ZT ---------------------------------
    # The host stores j-chunk (g + 4k) in Zb slot (4g + k), so transpose g
    # reads a contiguous slot group [4g, 4g+4) and PSUM band k ends up with
    # chunk g+4k: ZT band jb spans j [512 jb, 512 jb + 512) contiguously.
    ZT = sb.tile([P, NITILE * P], F32R, tag="ZT")
    for g in range(4):
        pz = tp.tile([P, P], F32, tag="tp")
        nc.tensor.transpose(
            pz[:], Zb[:, 4 * g : 4 * g + 4, :].rearrange("p c f -> p (c f)"), ident[:]
        )
        if g % 2 == 0:
            nc.scalar.copy(ZT[:, g * P : (g + 1) * P], pz[:])
        else:
            nc.vector.tensor_copy(ZT[:, g * P : (g + 1) * P], pz[:])

    # ---- main: matmul + sqrt + DMA out ------------------------------------
    for it in range(NITILE):
        for h in range(N // HALF):
            pm = pmp.tile([P, HALF], F32, tag="pm")
            for jj in range(HALF // 512):
                jb = (HALF // 512) * h + jj
                nc.tensor.matmul(
                    pm[:, jj * 512 : (jj + 1) * 512],
                    YTrep[32 * jb : 32 * jb + NF, it * P : (it + 1) * P],
                    ZT[32 * jb : 32 * jb + NF, :],
                    start=True,
                    stop=True,
                    tile_position=(32 * jb, 0),
                )
            ot = outp.tile([P, HALF], F32, tag="ot")
            nc.scalar.sqrt(ot[:], pm[:])
            nc.sync.dma_start(
                out=out_dram[it * P : (it + 1) * P, h * HALF : (h + 1) * HALF],
                in_=ot[:],
            )
            if it * 2 + h < 7:
                # keep the PE continuously busy between half-tiles so its
                # clock stays ramped (idle gaps re-throttle it)
                for w in range(NFILL):
                    wt = tp.tile([P, P], F32, tag="tp")
                    nc.tensor.transpose(wt[:], ident[:], ident[:])


def _shard_inputs(pred_coords, true_coords, pred_frames, true_frames, mask):
    """Host-side reformat into per-core DMA-friendly layouts."""
    pc = np.asarray(pred_coords, np.float32)
    tc = np.asarray(true_coords, np.float32)
    pf = np.asarray(pred_frames, np.float32)
    tf = np.asarray(true_frames, np.float32)
    mk = np.asarray(mask).astype(np.float32)

    in_maps = []
    for core in range(NCORES):
        b = core // (NCORES // B)
        i0 = (core % (NCORES // B)) * ISLICE
        # frames [128, set, c, pt, xyz] ; input frames are [n, xyz, pt].
        # j-chunk columns are permuted so Zb slot 4g+k holds chunk g+4k:
        # the device's contiguous transpose groups then produce partition
        # bands covering contiguous 512-j blocks.
        fr = np.stack([pf[b], tf[b]], axis=0)  # [2, n, 3xyz, 3pt]
        fr = fr.transpose(0, 1, 3, 2)  # [2, n, pt, xyz]
        fr = fr.reshape(2, NJCH, 128, 3, 3)[:, CHUNK_PERM]
        fr = fr.transpose(2, 0, 1, 3, 4)
        frames = np.ascontiguousarray(fr.reshape(128, -1))
        # coords [128, chunk, set, xyz]
        co = np.stack([pc[b, i0 : i0 + ISLICE], tc[b, i0 : i0 + ISLICE]], axis=1)
        co = co.reshape(NITILE, 128, 2, 3).transpose(1, 0, 2, 3)
        coords = np.ascontiguousarray(co.reshape(128, -1))
        maskj = np.ascontiguousarray(mk[b].reshape(NJCH, 128)[CHUNK_PERM].T)
        maski = np.ascontiguousarray(
            mk[b, i0 : i0 + ISLICE].reshape(NITILE, 128).T
        )
        allin = np.ascontiguousarray(
            np.concatenate([frames, coords, maskj, maski], axis=1)
        )
        in_maps.append({"allin": allin})
    return in_maps


def kernel(pred_coords, true_coords, pred_frames, true_frames, mask, _res=[]):
    nc = _build()
    in_maps = _shard_inputs(pred_coords, true_coords, pred_frames, true_frames, mask)
    res = run_bass_kernel_spmd(nc, in_maps, list(range(NCORES)))
    _res.clear()
    _res.append(res)
    out = np.empty((B, N, N), np.float32)
    for core in range(NCORES):
        b = core // (NCORES // B)
        i0 = (core % (NCORES // B)) * ISLICE
        out[b, i0 : i0 + ISLICE, :] = res.results[core]["out"]
    return out


if __name__ == "__main__":
    rng = np.random.default_rng(0)
    ins = {
        "pred_coords": rng.standard_normal((B, N, 3)).astype(np.float32),
        "true_coords": rng.standard_normal((B, N, 3)).astype(np.float32),
        "pred_frames": rng.standard_normal((B, N, 3, 3)).astype(np.float32),
        "true_frames": rng.standard_normal((B, N, 3, 3)).astype(np.float32),
        "mask": np.ones((B, N), bool),
    }
    out = kernel(**ins)
    print("out", out.shape, out.dtype, float(np.abs(out).max()))


# revision 16
# speedup vs baseline: 1.1366x; 1.0779x over previous
"""ComputeAlignmentError kernel for 8 TRN2 NeuronCores.

Math: for each batch b, pairwise alignment error
    err[i,j] = || Ep_j (pc_i - bp_j) - Et_j (tc_i - bt_j) + eps ||_2
where Ep/Et are orthonormal frame bases built from pred/true frames and
bp/bt are the frame origins.  Because Ep/Et are rotations, this collapses
into a rank-18 bilinear form  err^2[i,j] = Y[i] . Z[j]  with
    Y[i] = [1, |pc|^2, |tc|^2, pc, tc, vec(pc tc^T)] * mask_i     (18)
    Z[j] = [z0 + delta, 1, 1, -2(bp - R bt), -2(bt - R^T bp),
            -2 vec(R)] * mask_j                                   (18)
    R_j = Ep_j^T Et_j,  z0 = |bp|^2 + |bt|^2 - 2 bp.(R bt)
The 1e-8 eps terms of the reference are dropped (contribute ~1e-7 to
err^2; tolerance is 2e-2).  `delta` (~1e-2) keeps err^2 positive under
float32r matmul rounding so the ACT sqrt never sees a negative input;
it biases err by delta/(2 err) ~ 2e-3 absolute, well inside tolerance.
Masked entries stay exactly 0: the mask multiplies every Z/Y slot
(including z0+delta), so err^2 = 0 -> sqrt = 0.

Each core handles one (batch, 512-row i-slice): builds Z for all 2048 j
and Y for its 512 i on-chip, transposes both feature-major via the PE
(strided chunk grouping so each [18,512] rhs block lands on one 32-row
partition band), then per (i-tile, half): 2 f32r matmuls [18,128]x[18,512]
-> PSUM, one ACT sqrt pass PSUM->SBUF, one 512 KB DMA.  Dummy warm-up
transposes keep the PE clock-gate (HAM) warm through the feature phase.
"""

import sys

import numpy as np

sys.path.insert(0, "/opt/trn_rl_repo")

from contextlib import ExitStack

import concourse.bacc as bacc
import concourse.bass as bass
import concourse.tile as tile
from concourse import mybir
from concourse.bass_utils import run_bass_kernel_spmd
from concourse.masks import make_identity

F32 = mybir.dt.float32
F32R = mybir.dt.float32r
ALU = mybir.AluOpType

B, N = 2, 2048
NCORES = 8
ISLICE = N * B // NCORES  # 512 rows of i per core
NITILE = ISLICE // 128  # 4 i-tiles per core
NJCH = N // 128  # 16 j-chunks
NF = 18  # feature count K
FPAD = 32  # feature slot padding (partition-band alignment)
G = 2 * NJCH  # 32 (set, chunk) groups

DELTA = 1e-2  # err^2 positivity margin under f32r rounding
NWARM = 6  # PE warm-up transposes gated on the feature phase
NFILL = 1  # PE filler transposes between output half-tiles
HALF = 1024  # output processed in [128, HALF] half-tiles

# Zb slot q holds j-chunk CHUNK_PERM[q] = (q // 4) + 4 * (q % 4) (self-inverse)
CHUNK_PERM = [(q // 4) + 4 * (q % 4) for q in range(NJCH)]


def _build(nc_holder=[]):
    if nc_holder:
        return nc_holder[0]
    nc = bacc.Bacc(
        "TRN2",
        target_bir_lowering=False,
        debug=False,
        enable_asserts=True,
        num_devices=NCORES,
    )
    NIN = 2 * NJCH * 9 + NITILE * 6 + NJCH + NITILE  # 332
    all_in = nc.dram_tensor("allin", [128, NIN], F32, kind="ExternalInput").ap()
    out_dram = nc.dram_tensor("out", [ISLICE, N], F32, kind="ExternalOutput").ap()

    with tile.TileContext(nc) as tc, ExitStack() as ctx:
        _kernel_body(ctx, tc, out_dram, all_in)

    nc.compile()
    nc_holder.append(nc)
    return nc


def _kernel_body(ctx, tc, out_dram, all_in):
    nc = tc.nc
    P = 128
    sb = ctx.enter_context(tc.tile_pool(name="sb", bufs=1))
    outp = ctx.enter_context(tc.tile_pool(name="outp", bufs=3))
    tp = ctx.enter_context(tc.tile_pool(name="tp", bufs=2, space="PSUM"))
    pmp = ctx.enter_context(tc.tile_pool(name="pmp", bufs=3, space="PSUM"))

    # ---- one merged input DMA (single completion semaphore) ---------------
    NIN = 2 * NJCH * 9 + NITILE * 6 + NJCH + NITILE
    Ain = sb.tile([P, NIN], F32, tag="Ain")
    nc.sync.dma_start(out=Ain[:], in_=all_in[:])
    o0 = 0
    Ft = Ain[:, o0 : o0 + 2 * NJCH * 9].rearrange(
        "p (s c t x) -> p s c t x", s=2, c=NJCH, t=3
    )
    o0 += 2 * NJCH * 9
    Ct = Ain[:, o0 : o0 + NITILE * 6].rearrange("p (c s x) -> p c s x", c=NITILE, s=2)
    o0 += NITILE * 6
    Mj = Ain[:, o0 : o0 + NJCH]
    o0 += NJCH
    Mi = Ain[:, o0 : o0 + NITILE]

    # ---- t=0 setup: ACT table trigger, identity, PE warm-up ---------------
    scr = sb.tile([P, 2], F32, tag="scr")
    nc.gpsimd.memset(scr[:, 0:1], 1.0)
    # first ACT op is a sqrt -> loads sqrt_and_others once (covers square/
    # copy/identity too); runs under the input-DMA wait
    nc.scalar.sqrt(scr[:, 1:2], scr[:, 0:1])

    ident = sb.tile([P, P], F32, tag="ident")
    make_identity(nc, ident[:])

    # ---- early independent ops --------------------------------------------
    Zb = sb.tile([P, NJCH, FPAD], F32, tag="Zb")
    nc.gpsimd.tensor_copy(
        Zb[:, :, 1:3], Mj.unsqueeze(2).broadcast_to((P, NJCH, 2))
    )
    # z0 partials: [bp, bt]^2 -> H9[0:6] (needs only frames)
    bb = Ft[:, :, :, 1, :].transpose([0, 2, 1, 3])  # [p, c, set, xyz]
    H9 = sb.tile([P, NJCH, 9], F32, tag="H9")
    nc.scalar.square(H9[:, :, 0:6].rearrange("p c (s x) -> p c s x", s=2), bb)

    # ---- Y features (band-replicated for the 4 PE row groups) -------------
    YbR = sb.tile([P, NITILE, 4, FPAD], F32, tag="YbR")  # [p, chunk, band, f]
    Cm = sb.tile([P, NITILE, 2, 3], F32, tag="Cm")
    nc.gpsimd.tensor_mul(
        Cm[:], Ct, Mi.unsqueeze(2).unsqueeze(3).broadcast_to((P, NITILE, 2, 3))
    )
    sqc = sb.tile([P, NITILE, 2, 3], F32, tag="sqc")
    nc.gpsimd.tensor_mul(sqc[:], Cm[:], Ct)
    nc.gpsimd.tensor_copy(YbR[:, :, 0, 0:1], Mi.unsqueeze(2))
    nc.gpsimd.tensor_copy(
        YbR[:, :, 0, 3:9], Cm[:].rearrange("p c s x -> p c (s x)")
    )
    nc.vector.reduce_sum(
        YbR[:, :, 0, 1:3].unsqueeze(3), sqc[:], axis=mybir.AxisListType.X
    )
    nc.vector.tensor_mul(
        YbR[:, :, 0, 9:18].rearrange("p c (a b) -> p c a b", a=3),
        Cm[:, :, 0, :].unsqueeze(3).broadcast_to((P, NITILE, 3, 3)),
        Ct[:, :, 1, :].unsqueeze(2).broadcast_to((P, NITILE, 3, 3)),
    )
    nc.vector.tensor_copy(
        YbR[:, :, 1:4, 0:NF],
        YbR[:, :, 0, 0:NF].unsqueeze(2).broadcast_to((P, NITILE, 3, NF)),
    )

    # Y transposes: in_ = [p, (band, f)] per chunk -> PSUM bands all hold
    # that chunk's features; one [128,128] copy per chunk into YTrep
    YTrep = sb.tile([P, NITILE * P], F32R, tag="YTrep")
    for t in range(NITILE):
        pt = tp.tile([P, P], F32, tag="tp")
        nc.tensor.transpose(
            pt[:], YbR[:, t, :, :].rearrange("p b f -> p (b f)"), ident[:]
        )
        with tc.high_priority(offset=-100000):
            if t % 2 == 0:
                nc.scalar.copy(YTrep[:, t * P : (t + 1) * P], pt[:])
            else:
                nc.vector.tensor_copy(YTrep[:, t * P : (t + 1) * P], pt[:])

    # ---- frame bases (both sets, all j-chunks at once) --------------------
    Fg = Ft.rearrange("p s c t x -> p (s c) t x")  # [p, g, pt, xyz]
    w12 = sb.tile([P, G, 2, 3], F32, tag="w12")  # w1 = a - b, w2 = c - b
    nc.vector.tensor_sub(
        w12[:],
        Fg[:, :, 0::2, :],
        Fg[:, :, 1, :].unsqueeze(2).broadcast_to((P, G, 2, 3)),
    )
    sq1 = sb.tile([P, G, 2, 3], F32, tag="sq1")
    nc.vector.tensor_mul(sq1[:], w12[:], w12[:])
    n2 = sb.tile([P, G, 2], F32, tag="n2")
    nc.vector.reduce_sum(n2[:].unsqueeze(3), sq1[:], axis=mybir.AxisListType.X)
    nrm = sb.tile([P, G, 2], F32, tag="nrm")
    nc.scalar.sqrt(nrm[:], n2[:])
    rinv = sb.tile([P, G, 2], F32, tag="rinv")
    nc.vector.reciprocal_approx_fast(rinv[:], nrm[:])
    w12n = sb.tile([P, G, 2, 3], F32, tag="w12n")
    nc.vector.tensor_mul(
        w12n[:], w12[:], rinv[:].unsqueeze(3).broadcast_to((P, G, 2, 3))
    )

    e12p = sb.tile([P, G, 2, 3], F32, tag="e12p")
    nc.vector.tensor_add(e12p[:, :, 0, :], w12n[:, :, 0, :], w12n[:, :, 1, :])
    nc.vector.tensor_sub(e12p[:, :, 1, :], w12n[:, :, 1, :], w12n[:, :, 0, :])
    sq2 = sb.tile([P, G, 2, 3], F32, tag="sq2")
    nc.vector.tensor_mul(sq2[:], e12p[:], e12p[:])
    n2b = sb.tile([P, G, 2], F32, tag="n2b")
    nc.vector.reduce_sum(n2b[:].unsqueeze(3), sq2[:], axis=mybir.AxisListType.X)
    nrmb = sb.tile([P, G, 2], F32, tag="nrmb")
    nc.scalar.sqrt(nrmb[:], n2b[:])
    rinvb = sb.tile([P, G, 2], F32, tag="rinvb")
    nc.vector.reciprocal_approx_fast(rinvb[:], nrmb[:])

    # Est[p, g, k, xyz]: rows e1, e2 normalized; e3 = e1 x e2
    Est = sb.tile([P, G, 3, 3], F32, tag="Est")
    nc.vector.tensor_mul(
        Est[:, :, 0:2, :],
        e12p[:],
        rinvb[:].unsqueeze(3).broadcast_to((P, G, 2, 3)),
    )
    cbuf = sb.tile([P, G, 2, 6], F32, tag="cbuf")  # duplicated for rotations
    nc.vector.tensor_copy(cbuf[:, :, :, 0:3], Est[:, :, 0:2, :])
    nc.scalar.copy(cbuf[:, :, :, 3:6], Est[:, :, 0:2, :])
    mtmp = sb.tile([P, G, 2, 3], F32, tag="mtmp")
    nc.vector.tensor_mul(mtmp[:, :, 0, :], cbuf[:, :, 0, 1:4], cbuf[:, :, 1, 2:5])
    nc.vector.tensor_mul(mtmp[:, :, 1, :], cbuf[:, :, 0, 2:5], cbuf[:, :, 1, 1:4])
    nc.vector.tensor_sub(Est[:, :, 2, :], mtmp[:, :, 0, :], mtmp[:, :, 1, :])

    # ---- Z features -------------------------------------------------------
    Estv = Est[:].rearrange("p (s c) k x -> p s c k x", s=2)
    Ep = Estv[:, 0]  # [p, c, k, xyz]
    Et_ = Estv[:, 1]
    bp = Ft[:, 0, :, 1, :]  # [p, c, xyz]
    bt = Ft[:, 1, :, 1, :]

    # R[c, a, b] = sum_k Ep[c,k,a] * Et[c,k,b]; EtT materialized once so the
    # three muls read a contiguous in1 (transposed APs run ~2x slower on DVE)
    EtT = sb.tile([P, NJCH, 3, 3], F32, tag="EtT")  # [c, xyz, k]
    nc.vector.tensor_copy(EtT[:], Et_.transpose([0, 1, 3, 2]))
    prodR = sb.tile([P, NJCH, 9, 3], F32, tag="prodR")  # [c, (a b), k]
    for a in range(3):
        eng = nc.gpsimd if a == 2 else nc.vector
        eng.tensor_mul(
            prodR[:, :, 3 * a : 3 * a + 3, :],
            Ep[:, :, :, a].unsqueeze(2).broadcast_to((P, NJCH, 3, 3)),
            EtT[:],
        )
    # PE warm-up gated on prodR: sustained activity right before the real burst
    for w in range(NWARM):
        wt = tp.tile([P, P], F32, tag="tp")
        nc.tensor.transpose(
            wt[:], prodR[:].rearrange("p c f x -> p (c f x)")[:, 0:P], ident[:]
        )

    Rb = sb.tile([P, NJCH, 9], F32, tag="Rb")
    nc.vector.reduce_sum(Rb[:].unsqueeze(3), prodR[:], axis=mybir.AxisListType.X)
    Rbv = Rb[:].rearrange("p c (a b) -> p c a b", a=3)

    # Rbt[c,a] = sum_b R[c,a,b] bt[c,b] ; Rtbp[c,b] = sum_a R[c,a,b] bp[c,a]
    prodv = sb.tile([P, NJCH, 6, 3], F32, tag="prodv")
    nc.vector.tensor_mul(
        prodv[:, :, 0:3, :], Rbv, bt.unsqueeze(2).broadcast_to((P, NJCH, 3, 3))
    )
    nc.vector.tensor_mul(
        prodv[:, :, 3:6, :],
        Rbv.transpose([0, 1, 3, 2]),
        bp.unsqueeze(2).broadcast_to((P, NJCH, 3, 3)),
    )
    Rv = sb.tile([P, NJCH, 6], F32, tag="Rv")  # [Rbt | Rtbp]
    nc.vector.reduce_sum(Rv[:].unsqueeze(3), prodv[:], axis=mybir.AxisListType.X)

    Zb = Zb  # [P, NJCH, FPAD]; slots 1:3 already hold Mj
    t2 = sb.tile([P, NJCH, 2, 3], F32, tag="t2")
    nc.vector.tensor_sub(t2[:], bb, Rv[:].rearrange("p c (v x) -> p c v x", v=2))
    # zp|zt = -2 (b - Rv) * mask
    nc.vector.scalar_tensor_tensor(
        Zb[:, :, 3:9],
        t2[:].rearrange("p c s x -> p c (s x)"),
        -2.0,
        Mj.unsqueeze(2).broadcast_to((P, NJCH, 6)),
        op0=ALU.mult,
        op1=ALU.mult,
    )
    # -2 R * mask
    nc.vector.scalar_tensor_tensor(
        Zb[:, :, 9:18],
        Rb[:],
        -2.0,
        Mj.unsqueeze(2).broadcast_to((P, NJCH, 9)),
        op0=ALU.mult,
        op1=ALU.mult,
    )
    # z0 = |bp|^2 + |bt|^2 - 2 bp.Rbt ; slot0 = (z0 + delta) * mask
    nc.vector.scalar_tensor_tensor(
        H9[:, :, 6:9], bp, -2.0, Rv[:, :, 0:3], op0=ALU.mult, op1=ALU.mult
    )
    z0 = sb.tile([P, NJCH], F32, tag="z0")
    nc.vector.reduce_sum(z0[:].unsqueeze(2), H9[:], axis=mybir.AxisListType.X)
    nc.vector.scalar_tensor_tensor(
        Zb[:, :, 0:1],
        z0[:].unsqueeze(2),
        DELTA,
        Mj.unsqueeze(2),
        op0=ALU.add,
        op1=ALU.mult,
    )

    # ---- Z transposes -> band-coherent ZT ---------------------------------
    # The host stores j-chunk (g + 4k) in Zb slot (4g + k), so transpose g
    # reads a contiguous slot group [4g, 4g+4) and PSUM band k ends up with
    # chunk g+4k: ZT band jb spans j [512 jb, 512 jb + 512) contiguously.
    ZT = sb.tile([P, NITILE * P], F32R, tag="ZT")
    for g in range(4):
        pz = tp.tile([P, P], F32, tag="tp")
        nc.tensor.transpose(
            pz[:], Zb[:, 4 * g : 4 * g + 4, :].rearrange("p c f -> p (c f)"), ident[:]
        )
        if g % 2 == 0:
            nc.scalar.copy(ZT[:, g * P : (g + 1) * P], pz[:])
        else:
            nc.vector.tensor_copy(ZT[:, g * P : (g + 1) * P], pz[:])

    # ---- main: matmul + sqrt + DMA out ------------------------------------
    for it in range(NITILE):
        for h in range(N // HALF):
            pm = pmp.tile([P, HALF], F32, tag="pm")
            for jj in range(HALF // 512):
                jb = (HALF // 512) * h + jj
                nc.tensor.matmul(
                    pm[:, jj * 512 : (jj + 1) * 512],
                    YTrep[32 * jb : 32 * jb + NF, it * P : (it + 1) * P],
                    ZT[32 * jb : 32 * jb + NF, :],
                    start=True,
                    stop=True,
                    tile_position=(32 * jb, 0),
                )
            ot = outp.tile([P, HALF], F32, tag="ot")
            nc.scalar.sqrt(ot[:], pm[:])
            nc.sync.dma_start(
                out=out_dram[it * P : (it + 1) * P, h * HALF : (h + 1) * HALF],
                in_=ot[:],
            )
            if it * 2 + h < 7:
                # keep the PE continuously busy between half-tiles so its
                # clock stays ramped (idle gaps re-throttle it)
                for w in range(NFILL):
                    wt = tp.tile([P, P], F32, tag="tp")
                    nc.tensor.transpose(wt[:], ident[:], ident[:])


def _shard_inputs(pred_coords, true_coords, pred_frames, true_frames, mask):
    """Host-side reformat into per-core DMA-friendly layouts."""
    pc = np.asarray(pred_coords, np.float32)
    tc = np.asarray(true_coords, np.float32)
    pf = np.asarray(pred_frames, np.float32)
    tf = np.asarray(true_frames, np.float32)
    mk = np.asarray(mask).astype(np.float32)

    in_maps = []
    for core in range(NCORES):
        b = core // (NCORES // B)
        i0 = (core % (NCORES // B)) * ISLICE
        # frames [128, set, c, pt, xyz] ; input frames are [n, xyz, pt].
        # j-chunk columns are permuted so Zb slot 4g+k holds chunk g+4k:
        # the device's contiguous transpose groups then produce partition
        # bands covering contiguous 512-j blocks.
        fr = np.stack([pf[b], tf[b]], axis=0)  # [2, n, 3xyz, 3pt]
        fr = fr.transpose(0, 1, 3, 2)  # [2, n, pt, xyz]
        fr = fr.reshape(2, NJCH, 128, 3, 3)[:, CHUNK_PERM]
        fr = fr.transpose(2, 0, 1, 3, 4)
        frames = np.ascontiguousarray(fr.reshape(128, -1))
        # coords [128, chunk, set, xyz]
        co = np.stack([pc[b, i0 : i0 + ISLICE], tc[b, i0 : i0 + ISLICE]], axis=1)
        co = co.reshape(NITILE, 128, 2, 3).transpose(1, 0, 2, 3)
        coords = np.ascontiguousarray(co.reshape(128, -1))
        maskj = np.ascontiguousarray(mk[b].reshape(NJCH, 128)[CHUNK_PERM].T)
        maski = np.ascontiguousarray(
            mk[b, i0 : i0 + ISLICE].reshape(NITILE, 128).T
        )
        allin = np.ascontiguousarray(
            np.concatenate([frames, coords, maskj, maski], axis=1)
        )
        in_maps.append({"allin": allin})
    return in_maps


def kernel(pred_coords, true_coords, pred_frames, true_frames, mask, _res=[]):
    nc = _build()
    in_maps = _shard_inputs(pred_coords, true_coords, pred_frames, true_frames, mask)
    res = run_bass_kernel_spmd(nc, in_maps, list(range(NCORES)))
    _res.clear()
    _res.append(res)
    out = np.empty((B, N, N), np.float32)
    for core in range(NCORES):
        b = core // (NCORES // B)
        i0 = (core % (NCORES // B)) * ISLICE
        out[b, i0 : i0 + ISLICE, :] = res.results[core]["out"]
    return out


if __name__ == "__main__":
    rng = np.random.default_rng(0)
    ins = {
        "pred_coords": rng.standard_normal((B, N, 3)).astype(np.float32),
        "true_coords": rng.standard_normal((B, N, 3)).astype(np.float32),
        "pred_frames": rng.standard_normal((B, N, 3, 3)).astype(np.float32),
        "true_frames": rng.standard_normal((B, N, 3, 3)).astype(np.float32),
        "mask": np.ones((B, N), bool),
    }
    out = kernel(**ins)
    print("out", out.shape, out.dtype, float(np.abs(out).max()))
